# revision 24
# baseline (speedup 1.0000x reference)
"""Trainium2 Bass kernel for nn_CustomCNN (dense_cnn), v2.

Network (per image, 28x28 single channel):
  conv5x5(same) -> relu -> maxpool2     [28,28] -> [14,14]
  conv5x5(same) -> relu -> maxpool2     [14,14] -> [7,7]
  fc 49->128 + bias -> relu
  fc 128->10 + bias
  log_softmax

Strategy: pure data parallel over 8 NeuronCores (8192 images each).
v2 layout decisions (vs v1):
  - Host pre-transposes x to pixel-major tile form xt7 [7, 112, B] so the
    device does plain strided DMA loads (no DMA_TRANSPOSE on the sync queue).
  - Conv1 input tiles are chunk-aligned (rows 4m-2..4m+1), so every output
    chunk contracts exactly 2 tiles: 14 matmuls per 512-image group.
  - Pool1 = ACT relu-evict of the s=0 half (partition shift to base 0) +
    DVE max(SBUF, PSUM[64:]) (mixed-space TT allows different partition
    bases) + t-stage copy/max into a PACKED x2 layout [128, 2, 512] whose
    windows all start at 32-aligned partitions - no scatter DMAs.
  - Conv2 contracts the packed x2 in 2x2 matmuls; pool2 the same way.
"""

import os
import sys

import numpy as np

sys.path.insert(0, "/opt/trn_rl_repo")

import ml_dtypes

BF16 = ml_dtypes.bfloat16

# ---------------------------------------------------------------------------
# Problem constants (hardcoded per the harness contract)
# ---------------------------------------------------------------------------
B_TOTAL = 65536
N_CORES = 8
B_CORE = B_TOTAL // N_CORES          # 8192
NG = 512                             # images per group
N_GROUPS = B_CORE // NG              # 16
NSUB = NG // 128                     # 4 sub-chunks of 128 for fc2/log_softmax


# ---------------------------------------------------------------------------
# Host-side layout helpers
# ---------------------------------------------------------------------------

def _tile_home(p_flat):
    """Map input pixel p_flat (0..783) -> (col, row) in xt7 [7, 112, B].

    Tiles T_m = rows 4m-2..4m+1 (clipped): T_0 = px [0,56), T_m =
    [112m-56, 112m+56) for 1<=m<=6, T_7 = [728, 784).  Columns 0..5 hold
    T_1..T_6; column 6 holds [T_0 ; T_7]."""
    m = (p_flat + 56) // 112
    if m == 0:
        return 6, p_flat
    if m == 7:
        return 6, p_flat - 728 + 56
    return m - 1, p_flat - (112 * m - 56)


def conv1_windows():
    """x-stationary conv1 matmul windows: (w, m, range_idx, start_flag).

    Output o' = 128*rm + (f-28*rm)*4 + quad, f = i2*14+j2, rm = f//28
    (each 112-col range padded to 128 for PSUM bank alignment).  Tile m
    freshly writes range m (start=True) and accumulates range m-1."""
    out = []
    w = 0
    for m in range(8):
        if m >= 1:
            out.append((w, m, m - 1, False))
            w += 1
        if m <= 6:
            out.append((w, m, m, True))
            w += 1
    return out


def build_conv1_mats(k1):
    """wt1x [128, 14, 128]: x-stationary window weights (rhs operand)."""
    mats = np.zeros((14, 128, 128), np.float32)
    for w, m, rm, _ in conv1_windows():
        for co in range(112):
            f = 28 * rm + co // 4
            quad = co % 4
            i2, j2 = divmod(f, 14)
            a, b = divmod(quad, 2)
            i, j = 2 * i2 + a, 2 * j2 + b
            for di in range(5):
                for dj in range(5):
                    ii, jj = i + di - 2, j + dj - 2
                    if 0 <= ii < 28 and 0 <= jj < 28:
                        p_flat = 28 * ii + jj
                        if (p_flat + 56) // 112 == m:
                            col, row = _tile_home(p_flat)
                            mats[w][row, co] += k1[di, dj]
    return np.ascontiguousarray(mats.transpose(1, 0, 2))   # [128, 14, 128]


def col_for_tile(m):
    return 6 if m in (0, 7) else m - 1


def _x2_home(i2, j2):
    """x2 home of pool1 output pixel (i2, j2): padded-f\' space (224 rows,
    2 tiles of 112): f = i2*14+j2, fp = 32*(f//28) + f%28."""
    f = i2 * 14 + j2
    return f // 98, f % 98


def build_conv2_mats(k2):
    """wt2 [128, 4, 128]: slot s2*2+c contracts packed-x2 col c for out-row
    parity s2.  Out q2 = t2*64 + R*7 + u2 with i2o = 2R + s2, j2o = 2u2+t2."""
    mats = np.zeros((4, 98, 128), np.float32)
    for s2 in range(2):
        for R in range(7):
            i2o = 2 * R + s2
            for j2o in range(14):
                u2, t2 = divmod(j2o, 2)
                q2 = t2 * 64 + R * 7 + u2
                for di in range(5):
                    for dj in range(5):
                        i2, j2 = i2o + di - 2, j2o + dj - 2
                        if 0 <= i2 < 14 and 0 <= j2 < 14:
                            c, p2 = _x2_home(i2, j2)
                            mats[s2 * 2 + c][p2, q2] += k2[di, dj]
    return np.ascontiguousarray(mats.transpose(1, 0, 2))   # [98, 4, 128]


def build_host_weights(conv1_kernel, conv2_kernel, fc1_w, fc1_b, fc2_w, fc2_b):
    w1 = build_conv1_mats(np.asarray(conv1_kernel, np.float32))
    w2 = build_conv2_mats(np.asarray(conv2_kernel, np.float32))
    return {
        "wt1": w1.astype(BF16),                               # [112, 14, 128]
        "wt2": w2.astype(BF16),                               # [128, 4, 128]
        "fc1t": np.ascontiguousarray(np.asarray(fc1_w, np.float32).T).astype(BF16),  # [49, 128]
        "fc1b": np.asarray(fc1_b, np.float32).reshape(128, 1).copy(),
        "fc2t": np.ascontiguousarray(np.asarray(fc2_w, np.float32).T).astype(BF16),  # [128, 10]
        "fc2b": np.tile(np.asarray(fc2_b, np.float32).reshape(1, 10), (128, 1)),
        "ident": np.eye(128, dtype=np.float32).astype(BF16),
    }


def build_xt7(xbf):
    """xbf [B, 784] bf16 -> xt7 [7, 128, B] bf16 (pixel-major tile form,
    rows 112:128 zero-padded so conv1 lhsT is full 128 rows -> FWL)."""
    B = xbf.shape[0]
    xt = np.ascontiguousarray(xbf.T)                         # [784, B]
    out = np.zeros((7, 128, B), dtype=BF16)
    for m in range(1, 7):
        out[m - 1, 0:112] = xt[112 * m - 56:112 * m + 56]
    out[6, 0:56] = xt[0:56]
    out[6, 56:112] = xt[728:784]
    return out


# ---------------------------------------------------------------------------
# Pure-numpy emulation of the device pipeline (layout validation / debug)
# ---------------------------------------------------------------------------

def emulate_pipeline(x, hw, n_images=512):
    """Exact device dataflow in numpy (bf16-rounded matmul inputs, fp32
    accumulation) for n_images. Returns [n_images, 10] float32."""
    w1 = hw["wt1"].astype(np.float32)        # [128, 14, 128]
    w2 = hw["wt2"].astype(np.float32)        # [128, 4, 128]
    xbf = np.asarray(x, np.float32).reshape(-1, 784)[:n_images].astype(BF16)
    xt7 = build_xt7(xbf).astype(np.float32)  # [7, 112, B]

    # conv1 x-stationary: ps1 [B, 896] (7 ranges x 128), pool via quad-max
    ps1 = np.zeros((n_images, 896), np.float32)
    for w, m, rm, start in conv1_windows():
        ps1[:, 128 * rm:128 * rm + 128] += xt7[col_for_tile(m)].T @ w1[:, w, :]
    x2i = ps1.reshape(n_images, 7, 32, 4)[:, :, 0:28, :].max(axis=3)
    x2i = x2i.reshape(n_images, 196).astype(BF16).astype(np.float32)
    x2 = np.maximum(x2i.T, 0.0).astype(BF16).astype(np.float32)   # [196, B]

    # conv2
    ps2 = np.zeros((128, 2, n_images), np.float32)
    for s2 in range(2):
        for c in range(2):
            ps2[:, s2] += w2[:, s2 * 2 + c, :].T @ x2[98 * c:98 * c + 98]
    c2a = np.maximum(ps2[:, 0], 0.0).astype(BF16).astype(np.float32)
    m2 = np.maximum(c2a, ps2[:, 1]).astype(BF16).astype(np.float32)
    x3 = np.maximum(m2[0:49], m2[64:113]).astype(BF16).astype(np.float32)

    f1 = hw["fc1t"].astype(np.float32).T @ x3 + hw["fc1b"]          # [128, B]
    h = np.maximum(f1, 0.0).astype(BF16).astype(np.float32)
    logits = (h.T @ hw["fc2t"].astype(np.float32)) + hw["fc2b"][0]  # [B, 10]
    e = np.exp(logits)
    return (logits - np.log(e.sum(1, keepdims=True))).astype(np.float32)


# ---------------------------------------------------------------------------
# Bass kernel
# ---------------------------------------------------------------------------

def build_bass_kernel(n_groups=N_GROUPS):
    import concourse.bass as bass
    import concourse.tile as tile
    from concourse import bacc, mybir

    f32 = mybir.dt.float32
    bf16 = mybir.dt.bfloat16
    AF = mybir.ActivationFunctionType
    OP = mybir.AluOpType

    nc = bacc.Bacc("TRN2", target_bir_lowering=False, debug=False,
                   num_devices=N_CORES)

    b_core = n_groups * NG
    xt7 = nc.dram_tensor("xt7", [7, 128, b_core], bf16, kind="ExternalInput").ap()
    wt1 = nc.dram_tensor("wt1", [128, 14, 128], bf16, kind="ExternalInput").ap()
    ident = nc.dram_tensor("ident", [128, 128], bf16, kind="ExternalInput").ap()
    wt2 = nc.dram_tensor("wt2", [98, 4, 128], bf16, kind="ExternalInput").ap()
    fc1t = nc.dram_tensor("fc1t", [49, 128], bf16, kind="ExternalInput").ap()
    fc1b = nc.dram_tensor("fc1b", [128, 1], f32, kind="ExternalInput").ap()
    fc2t = nc.dram_tensor("fc2t", [128, 10], bf16, kind="ExternalInput").ap()
    fc2b = nc.dram_tensor("fc2b", [128, 10], f32, kind="ExternalInput").ap()
    y = nc.dram_tensor("y", [128, n_groups, NSUB, 10], f32, kind="ExternalOutput").ap()

    with tile.TileContext(nc) as tc:
        with (
            tc.tile_pool(name="wpool", bufs=1) as wpool,
            tc.tile_pool(name="inp", bufs=4) as inp,
            tc.tile_pool(name="work", bufs=6) as work,
            tc.tile_pool(name="hpool", bufs=6) as hpool,
            tc.tile_pool(name="outp", bufs=1) as outp,
            tc.tile_pool(name="psp", bufs=3, space="PSUM") as psp,
            tc.tile_pool(name="psl2", bufs=2, space="PSUM") as psl2,
        ):
            # ---- load weights once ----
            w1sb = wpool.tile([128, 14, 128], bf16)
            nc.sync.dma_start(w1sb, wt1)
            w2sb = wpool.tile([98, 4, 128], bf16)
            idsb = wpool.tile([128, 128], bf16)
            nc.sync.dma_start(idsb, ident)
            nc.sync.dma_start(w2sb, wt2)
            f1tsb = wpool.tile([49, 128], bf16)
            nc.sync.dma_start(f1tsb, fc1t)
            f1bsb = wpool.tile([128, 1], f32)
            nc.sync.dma_start(f1bsb, fc1b)
            f2tsb = wpool.tile([128, 10], bf16)
            nc.sync.dma_start(f2tsb, fc2t)
            f2bsb = wpool.tile([128, 10], f32)
            nc.sync.dma_start(f2bsb, fc2b)

            x2_all = outp.tile([98, n_groups, 2, NG], bf16)
            x3_all = outp.tile([49, n_groups, NG], bf16)

            xt7_v = xt7.rearrange("m p (g b) -> p m g b", g=n_groups)
            y_v = y

            # ================= phase 1: conv1 + pool1 =================
            for g in range(n_groups):
                xp = inp.tile([128, 7, NG], bf16, tag="xp")
                nc.scalar.dma_start(xp, xt7_v[:, :, g, :])

                # x-stationary conv1: out [img, o'] in psum, pool via
                # free-dim quad reduce, PE-transpose back to pixel-major
                x2i = work.tile([128, 4, 196], bf16, tag="x2i")
                for blk in range(4):
                    ps1 = psp.tile([128, 896], f32, tag="ps")
                    for w, m, rm, start in conv1_windows():
                        nc.tensor.matmul(ps1[:, 128 * rm:128 * rm + 128],
                                         xp[:, col_for_tile(m),
                                            blk * 128:(blk + 1) * 128],
                                         w1sb[:, w, :],
                                         start=start, stop=(not start))
                    nc.vector.tensor_reduce(
                        x2i[:, blk, :],
                        ps1.rearrange("p (m fl q) -> p m fl q",
                                      m=7, q=4)[:, :, 0:28, :],
                        mybir.AxisListType.X, OP.max)
                psT = psp.tile([98, 2, 4, 128], bf16, tag="ps")
                for blk in range(4):
                    for ct in range(2):
                        nc.tensor.transpose(psT[:, ct, blk, :],
                                            x2i[:, blk,
                                                98 * ct:98 * ct + 98],
                                            idsb)
                nc.scalar.activation(x2_all[:, g, :, :], psT, AF.Relu)

            # ================= phase 2: conv2 + pool2 =================
            for g in range(n_groups):
                ps2 = psp.tile([128, 2, NG], f32, tag="ps")
                for s2 in range(2):
                    for c in range(2):
                        nc.tensor.matmul(ps2[:, s2, :],
                                         w2sb[:, s2 * 2 + c, :],
                                         x2_all[:, g, c, :],
                                         start=(c == 0), stop=(c == 1))
                c2a = work.tile([128, NG], bf16, tag="c2a")
                nc.scalar.activation(c2a, ps2[:, 0, :], AF.Relu)
                c2b = work.tile([128, NG], bf16, tag="c2b")
                nc.scalar.activation(c2b, ps2[:, 1, :], AF.Relu)
                m2 = work.tile([128, NG], bf16, tag="m2")
                nc.vector.tensor_tensor(m2, c2a, c2b, OP.max)
                m2b = work.tile([64, NG], bf16, tag="m2b")
                nc.sync.dma_start(m2b, m2[64:128, :])
                nc.vector.tensor_tensor(x3_all[:, g, :], m2[0:49, :],
                                        m2b[0:49, :], OP.max)

            # ========== phase 3: fc1 + fc2 + log_softmax (4-group batches) ==
            for q in range(n_groups // 4):
                hs = []
                for j in range(4):
                    g = 4 * q + j
                    psf = psp.tile([128, NG], f32, tag="ps")
                    nc.tensor.matmul(psf, f1tsb, x3_all[:, g, :],
                                     start=True, stop=True)
                    h = hpool.tile([128, NG], bf16, tag="h")
                    nc.scalar.activation(h, psf, AF.Relu, bias=f1bsb[:, 0:1])
                    hs.append(h)

                psl = psl2.tile([128, 4, NSUB, 10], f32, tag="fc2")
                for j in range(4):
                    for u in range(NSUB):
                        nc.tensor.matmul(psl[:, j, u, :],
                                         hs[j][:, u * 128:(u + 1) * 128],
                                         f2tsb, start=True, stop=True)

                t2b = hpool.tile([128, 4, NSUB, 10], f32, tag="t2")
                nc.vector.tensor_tensor(
                    t2b, psl,
                    f2bsb[:, None, None, :].to_broadcast((128, 4, NSUB, 10)),
                    OP.add)
                e = work.tile([128, 4, NSUB, 10], f32, tag="e")
                nc.scalar.activation(e, t2b, AF.Exp)
                ssum = work.tile([128, 4, NSUB], f32, tag="ssum")
                nc.vector.tensor_reduce(ssum, e, mybir.AxisListType.X, OP.add)
                lg = work.tile([128, 4, NSUB], f32, tag="lg")
                nc.scalar.activation(lg, ssum, AF.Ln)
                ob = hpool.tile([128, 4, NSUB, 10], f32, tag="ob")
                nc.vector.tensor_tensor(
                    ob, t2b,
                    lg[:, :, :, None].to_broadcast((128, 4, NSUB, 10)),
                    OP.subtract)
                nc.sync.dma_start(y_v[:, 4 * q:4 * q + 4, :, :], ob)

    nc.compile()
    return nc


# ---------------------------------------------------------------------------
# Entry point
# ---------------------------------------------------------------------------

_CACHE = {}


def _install_ntff_hook():
    """Shim antenv.axon_hooks (absent on this image) with the ctypes hook
    from trn_agent_boot so run_bass_kernel_spmd(trace=True) can profile."""
    import types
    if "antenv.axon_hooks" in sys.modules:
        return
    try:
        from trn_agent_boot.trn_boot import _ntff_profile_via_ctypes
        hook = _ntff_profile_via_ctypes("/opt/axon/libaxon_pjrt.so")
    except Exception as e:
        print(f"ntff hook unavailable: {e}", file=sys.stderr)
        return
    if hook is None:
        return
    import antenv
    mod = types.ModuleType("antenv.axon_hooks")
    mod.get_axon_ntff_profile_hook = lambda: hook
    mod.set_axon_ntff_profile_hook = lambda h: None
    sys.modules["antenv.axon_hooks"] = mod
    antenv.axon_hooks = mod


def kernel(x, conv1_kernel, conv2_kernel, fc1_w, fc1_b, fc2_w, fc2_b):
    from concourse.bass_utils import run_bass_kernel_spmd

    hw = build_host_weights(conv1_kernel, conv2_kernel, fc1_w, fc1_b,
                            fc2_w, fc2_b)

    key = "nc"
    if key not in _CACHE:
        _CACHE[key] = build_bass_kernel()
    nc = _CACHE[key]

    xbf = np.asarray(x, np.float32).reshape(B_TOTAL, 784).astype(BF16)
    shared = {k: hw[k] for k in ("wt1", "wt2", "fc1t", "fc1b", "fc2t", "fc2b", "ident")}
    in_maps = []
    for c in range(N_CORES):
        m = dict(shared)
        m["xt7"] = build_xt7(xbf[c * B_CORE:(c + 1) * B_CORE])
        in_maps.append(m)

    trace = os.environ.get("KERNEL_TRACE", "0") == "1"
    if trace:
        _install_ntff_hook()
    res = run_bass_kernel_spmd(nc, in_maps, core_ids=list(range(N_CORES)),
                               trace=trace)
    if trace and res.exec_time_ns is not None:
        print(f"HW exec time: {res.exec_time_ns} ns", file=sys.stderr)
        _CACHE["exec_time_ns"] = res.exec_time_ns

    outs = []
    for r in res.results:
        yc = r["y"]                          # [128, n_groups, NSUB, 10]
        outs.append(np.ascontiguousarray(
            yc.transpose(1, 2, 0, 3).reshape(B_CORE, 10)))
    return np.concatenate(outs, axis=0)


# revision 25
# speedup vs baseline: 1.1049x; 1.1049x over previous
"""Trainium2 Bass kernel for nn_CustomCNN (dense_cnn), v2.

Network (per image, 28x28 single channel):
  conv5x5(same) -> relu -> maxpool2     [28,28] -> [14,14]
  conv5x5(same) -> relu -> maxpool2     [14,14] -> [7,7]
  fc 49->128 + bias -> relu
  fc 128->10 + bias
  log_softmax

Strategy: pure data parallel over 8 NeuronCores (8192 images each).
v2 layout decisions (vs v1):
  - Host pre-transposes x to pixel-major tile form xt7 [7, 112, B] so the
    device does plain strided DMA loads (no DMA_TRANSPOSE on the sync queue).
  - Conv1 input tiles are chunk-aligned (rows 4m-2..4m+1), so every output
    chunk contracts exactly 2 tiles: 14 matmuls per 512-image group.
  - Pool1 = ACT relu-evict of the s=0 half (partition shift to base 0) +
    DVE max(SBUF, PSUM[64:]) (mixed-space TT allows different partition
    bases) + t-stage copy/max into a PACKED x2 layout [128, 2, 512] whose
    windows all start at 32-aligned partitions - no scatter DMAs.
  - Conv2 contracts the packed x2 in 2x2 matmuls; pool2 the same way.
"""

import os
import sys

import numpy as np

sys.path.insert(0, "/opt/trn_rl_repo")

import ml_dtypes

BF16 = ml_dtypes.bfloat16

# ---------------------------------------------------------------------------
# Problem constants (hardcoded per the harness contract)
# ---------------------------------------------------------------------------
B_TOTAL = 65536
N_CORES = 8
B_CORE = B_TOTAL // N_CORES          # 8192
NG = 512                             # images per group
N_GROUPS = B_CORE // NG              # 16
NSUB = NG // 128                     # 4 sub-chunks of 128 for fc2/log_softmax


# ---------------------------------------------------------------------------
# Host-side layout helpers
# ---------------------------------------------------------------------------

def _tile_home(p_flat):
    """Map input pixel p_flat (0..783) -> (col, row) in xt7 [7, 112, B].

    Tiles T_m = rows 4m-2..4m+1 (clipped): T_0 = px [0,56), T_m =
    [112m-56, 112m+56) for 1<=m<=6, T_7 = [728, 784).  Columns 0..5 hold
    T_1..T_6; column 6 holds [T_0 ; T_7]."""
    m = (p_flat + 56) // 112
    if m == 0:
        return 6, p_flat
    if m == 7:
        return 6, p_flat - 728 + 56
    return m - 1, p_flat - (112 * m - 56)


def conv1_windows():
    """x-stationary conv1 matmul windows: (w, m, range_idx, start_flag).

    Output o' = 128*rm + (f-28*rm)*4 + quad, f = i2*14+j2, rm = f//28
    (each 112-col range padded to 128 for PSUM bank alignment).  Tile m
    freshly writes range m (start=True) and accumulates range m-1."""
    out = []
    w = 0
    for m in range(8):
        if m >= 1:
            out.append((w, m, m - 1, False))
            w += 1
        if m <= 6:
            out.append((w, m, m, True))
            w += 1
    return out


def build_conv1_mats(k1):
    """wt1x [128, 14, 128]: x-stationary window weights (rhs operand)."""
    mats = np.zeros((14, 128, 128), np.float32)
    for w, m, rm, _ in conv1_windows():
        for co in range(112):
            f = 28 * rm + co // 4
            quad = co % 4
            i2, j2 = divmod(f, 14)
            a, b = divmod(quad, 2)
            i, j = 2 * i2 + a, 2 * j2 + b
            for di in range(5):
                for dj in range(5):
                    ii, jj = i + di - 2, j + dj - 2
                    if 0 <= ii < 28 and 0 <= jj < 28:
                        p_flat = 28 * ii + jj
                        if (p_flat + 56) // 112 == m:
                            col, row = _tile_home(p_flat)
                            mats[w][row, co] += k1[di, dj]
    return np.ascontiguousarray(mats.transpose(1, 0, 2))   # [128, 14, 128]


def col_for_tile(m):
    return 6 if m in (0, 7) else m - 1


def _x2_home(i2, j2):
    """x2 home of pool1 output pixel (i2, j2): padded-f\' space (224 rows,
    2 tiles of 112): f = i2*14+j2, fp = 32*(f//28) + f%28."""
    f = i2 * 14 + j2
    return f // 98, f % 98


def build_conv2_mats(k2):
    """wt2 [128, 4, 128]: slot s2*2+c contracts packed-x2 col c for out-row
    parity s2.  Out q2 = t2*64 + R*7 + u2 with i2o = 2R + s2, j2o = 2u2+t2."""
    mats = np.zeros((4, 98, 128), np.float32)
    for s2 in range(2):
        for R in range(7):
            i2o = 2 * R + s2
            for j2o in range(14):
                u2, t2 = divmod(j2o, 2)
                q2 = t2 * 64 + R * 7 + u2
                for di in range(5):
                    for dj in range(5):
                        i2, j2 = i2o + di - 2, j2o + dj - 2
                        if 0 <= i2 < 14 and 0 <= j2 < 14:
                            c, p2 = _x2_home(i2, j2)
                            mats[s2 * 2 + c][p2, q2] += k2[di, dj]
    return np.ascontiguousarray(mats.transpose(1, 0, 2))   # [98, 4, 128]


def build_host_weights(conv1_kernel, conv2_kernel, fc1_w, fc1_b, fc2_w, fc2_b):
    w1 = build_conv1_mats(np.asarray(conv1_kernel, np.float32))
    w2 = build_conv2_mats(np.asarray(conv2_kernel, np.float32))
    return {
        "wt1": w1.astype(BF16),                               # [112, 14, 128]
        "wt2": w2.astype(BF16),                               # [128, 4, 128]
        "fc1t": np.ascontiguousarray(np.asarray(fc1_w, np.float32).T).astype(BF16),  # [49, 128]
        "fc1b": np.asarray(fc1_b, np.float32).reshape(128, 1).copy(),
        "fc2t": np.ascontiguousarray(np.asarray(fc2_w, np.float32).T).astype(BF16),  # [128, 10]
        "fc2b": np.tile(np.asarray(fc2_b, np.float32).reshape(1, 10), (128, 1)),
        "ident": np.eye(128, dtype=np.float32).astype(BF16),
    }


def build_xt7(xbf):
    """xbf [B, 784] bf16 -> xt7 [7, 128, B] bf16 (pixel-major tile form,
    rows 112:128 zero-padded so conv1 lhsT is full 128 rows -> FWL)."""
    B = xbf.shape[0]
    xt = np.ascontiguousarray(xbf.T)                         # [784, B]
    out = np.zeros((7, 128, B), dtype=BF16)
    for m in range(1, 7):
        out[m - 1, 0:112] = xt[112 * m - 56:112 * m + 56]
    out[6, 0:56] = xt[0:56]
    out[6, 56:112] = xt[728:784]
    return out


# ---------------------------------------------------------------------------
# Pure-numpy emulation of the device pipeline (layout validation / debug)
# ---------------------------------------------------------------------------

def emulate_pipeline(x, hw, n_images=512):
    """Exact device dataflow in numpy (bf16-rounded matmul inputs, fp32
    accumulation) for n_images. Returns [n_images, 10] float32."""
    w1 = hw["wt1"].astype(np.float32)        # [128, 14, 128]
    w2 = hw["wt2"].astype(np.float32)        # [128, 4, 128]
    xbf = np.asarray(x, np.float32).reshape(-1, 784)[:n_images].astype(BF16)
    xt7 = build_xt7(xbf).astype(np.float32)  # [7, 112, B]

    # conv1 x-stationary: ps1 [B, 896] (7 ranges x 128), pool via quad-max
    ps1 = np.zeros((n_images, 896), np.float32)
    for w, m, rm, start in conv1_windows():
        ps1[:, 128 * rm:128 * rm + 128] += xt7[col_for_tile(m)].T @ w1[:, w, :]
    x2i = ps1.reshape(n_images, 7, 32, 4)[:, :, 0:28, :].max(axis=3)
    x2i = x2i.reshape(n_images, 196).astype(BF16).astype(np.float32)
    x2 = np.maximum(x2i.T, 0.0).astype(BF16).astype(np.float32)   # [196, B]

    # conv2
    ps2 = np.zeros((128, 2, n_images), np.float32)
    for s2 in range(2):
        for c in range(2):
            ps2[:, s2] += w2[:, s2 * 2 + c, :].T @ x2[98 * c:98 * c + 98]
    c2a = np.maximum(ps2[:, 0], 0.0).astype(BF16).astype(np.float32)
    m2 = np.maximum(c2a, ps2[:, 1]).astype(BF16).astype(np.float32)
    x3 = np.maximum(m2[0:49], m2[64:113]).astype(BF16).astype(np.float32)

    f1 = hw["fc1t"].astype(np.float32).T @ x3 + hw["fc1b"]          # [128, B]
    h = np.maximum(f1, 0.0).astype(BF16).astype(np.float32)
    logits = (h.T @ hw["fc2t"].astype(np.float32)) + hw["fc2b"][0]  # [B, 10]
    e = np.exp(logits)
    return (logits - np.log(e.sum(1, keepdims=True))).astype(np.float32)


# ---------------------------------------------------------------------------
# Bass kernel
# ---------------------------------------------------------------------------

def build_bass_kernel(n_groups=N_GROUPS):
    import concourse.bass as bass
    import concourse.tile as tile
    from concourse import bacc, mybir

    f32 = mybir.dt.float32
    bf16 = mybir.dt.bfloat16
    AF = mybir.ActivationFunctionType
    OP = mybir.AluOpType

    nc = bacc.Bacc("TRN2", target_bir_lowering=False, debug=False,
                   num_devices=N_CORES)

    b_core = n_groups * NG
    xt7 = nc.dram_tensor("xt7", [7, 128, b_core], bf16, kind="ExternalInput").ap()
    wt1 = nc.dram_tensor("wt1", [128, 14, 128], bf16, kind="ExternalInput").ap()
    ident = nc.dram_tensor("ident", [128, 128], bf16, kind="ExternalInput").ap()
    wt2 = nc.dram_tensor("wt2", [98, 4, 128], bf16, kind="ExternalInput").ap()
    fc1t = nc.dram_tensor("fc1t", [49, 128], bf16, kind="ExternalInput").ap()
    fc1b = nc.dram_tensor("fc1b", [128, 1], f32, kind="ExternalInput").ap()
    fc2t = nc.dram_tensor("fc2t", [128, 10], bf16, kind="ExternalInput").ap()
    fc2b = nc.dram_tensor("fc2b", [128, 10], f32, kind="ExternalInput").ap()
    y = nc.dram_tensor("y", [128, n_groups, NSUB, 10], f32, kind="ExternalOutput").ap()

    with tile.TileContext(nc) as tc:
        with (
            tc.tile_pool(name="wpool", bufs=1) as wpool,
            tc.tile_pool(name="inp", bufs=4) as inp,
            tc.tile_pool(name="work", bufs=6) as work,
            tc.tile_pool(name="hpool", bufs=6) as hpool,
            tc.tile_pool(name="outp", bufs=1) as outp,
            tc.tile_pool(name="psp", bufs=3, space="PSUM") as psp,
            tc.tile_pool(name="psl2", bufs=2, space="PSUM") as psl2,
        ):
            # ---- load weights once ----
            w1sb = wpool.tile([128, 14, 128], bf16)
            nc.sync.dma_start(w1sb, wt1)
            w2sb = wpool.tile([98, 4, 128], bf16)
            idsb = wpool.tile([128, 128], bf16)
            nc.sync.dma_start(idsb, ident)
            nc.sync.dma_start(w2sb, wt2)
            f1tsb = wpool.tile([49, 128], bf16)
            nc.sync.dma_start(f1tsb, fc1t)
            f1bsb = wpool.tile([128, 1], f32)
            nc.sync.dma_start(f1bsb, fc1b)
            f2tsb = wpool.tile([128, 10], bf16)
            nc.sync.dma_start(f2tsb, fc2t)
            f2bsb = wpool.tile([128, 10], f32)
            nc.sync.dma_start(f2bsb, fc2b)

            out_sb = outp.tile([128, n_groups, NSUB, 10], f32)
            t2_all = outp.tile([128, n_groups, NSUB, 10], f32)
            ssum_all = outp.tile([128, n_groups, NSUB], f32)
            x2_all = outp.tile([98, n_groups, 2, NG], bf16)
            x3_all = outp.tile([49, n_groups, NG], bf16)

            xt7_v = xt7.rearrange("m p (g b) -> p m g b", g=n_groups)

            # ================= phase 1: conv1 + pool1 =================
            for g in range(n_groups):
                xp = inp.tile([128, 7, NG], bf16, tag="xp")
                nc.scalar.dma_start(xp, xt7_v[:, :, g, :])

                # x-stationary conv1: out [img, o'] in psum, pool via
                # free-dim quad reduce, PE-transpose back to pixel-major
                x2i = work.tile([128, 4, 196], bf16, tag="x2i")
                for blk in range(4):
                    ps1 = psp.tile([128, 896], f32, tag="ps")
                    for w, m, rm, start in conv1_windows():
                        nc.tensor.matmul(ps1[:, 128 * rm:128 * rm + 128],
                                         xp[:, col_for_tile(m),
                                            blk * 128:(blk + 1) * 128],
                                         w1sb[:, w, :],
                                         start=start, stop=(not start))
                    nc.vector.tensor_reduce(
                        x2i[:, blk, :],
                        ps1.rearrange("p (m fl q) -> p m fl q",
                                      m=7, q=4)[:, :, 0:28, :],
                        mybir.AxisListType.X, OP.max)
                psT = psp.tile([98, 2, 4, 128], bf16, tag="ps")
                for blk in range(4):
                    for ct in range(2):
                        nc.tensor.transpose(psT[:, ct, blk, :],
                                            x2i[:, blk,
                                                98 * ct:98 * ct + 98],
                                            idsb)
                nc.scalar.activation(x2_all[:, g, :, :], psT, AF.Relu)

            # ================= phase 2: conv2 + pool2 =================
            for g in range(n_groups):
                ps2 = psp.tile([128, 2, NG], f32, tag="ps")
                for s2 in range(2):
                    for c in range(2):
                        nc.tensor.matmul(ps2[:, s2, :],
                                         w2sb[:, s2 * 2 + c, :],
                                         x2_all[:, g, c, :],
                                         start=(c == 0), stop=(c == 1))
                c2a = work.tile([128, NG], bf16, tag="c2a")
                nc.scalar.activation(c2a, ps2[:, 0, :], AF.Relu)
                m2 = work.tile([128, NG], bf16, tag="m2")
                nc.vector.tensor_tensor(m2, c2a, ps2[:, 1, :], OP.max)
                m2b = work.tile([64, NG], bf16, tag="m2b")
                nc.sync.dma_start(m2b, m2[64:128, :])
                nc.vector.tensor_tensor(x3_all[:, g, :], m2[0:49, :],
                                        m2b[0:49, :], OP.max)

            # ========== phase 3: fc1 + fc2 + log_softmax (4-group batches) ==
            for q in range(n_groups // 4):
                hs = []
                for j in range(4):
                    g = 4 * q + j
                    psf = psp.tile([128, NG], f32, tag="ps")
                    nc.tensor.matmul(psf, f1tsb, x3_all[:, g, :],
                                     start=True, stop=True)
                    h = hpool.tile([128, NG], bf16, tag="h")
                    nc.scalar.activation(h, psf, AF.Relu, bias=f1bsb[:, 0:1])
                    hs.append(h)

                psl = psl2.tile([128, 4, NSUB, 10], f32, tag="fc2")
                for j in range(4):
                    for u in range(NSUB):
                        nc.tensor.matmul(psl[:, j, u, :],
                                         hs[j][:, u * 128:(u + 1) * 128],
                                         f2tsb, start=True, stop=True)

                t2s = t2_all[:, 4 * q:4 * q + 4, :, :]
                nc.vector.tensor_tensor(
                    t2s, psl,
                    f2bsb[:, None, None, :].to_broadcast((128, 4, NSUB, 10)),
                    OP.add)
                e = work.tile([128, 4, NSUB, 10], f32, tag="e")
                nc.scalar.activation(e, t2s, AF.Exp)
                nc.vector.tensor_reduce(ssum_all[:, 4 * q:4 * q + 4, :], e,
                                        mybir.AxisListType.X, OP.add)

            # ---- batched log + final subtract + store ----
            lg_all = outp.tile([128, n_groups, NSUB], f32)
            nc.scalar.activation(lg_all, ssum_all, AF.Ln)
            nc.vector.tensor_tensor(
                out_sb, t2_all,
                lg_all[:, :, :, None].to_broadcast((128, n_groups, NSUB, 10)),
                OP.subtract)
            nc.sync.dma_start(y, out_sb)

    nc.compile()
    return nc


# ---------------------------------------------------------------------------
# Entry point
# ---------------------------------------------------------------------------

_CACHE = {}


def _install_ntff_hook():
    """Shim antenv.axon_hooks (absent on this image) with the ctypes hook
    from trn_agent_boot so run_bass_kernel_spmd(trace=True) can profile."""
    import types
    if "antenv.axon_hooks" in sys.modules:
        return
    try:
        from trn_agent_boot.trn_boot import _ntff_profile_via_ctypes
        hook = _ntff_profile_via_ctypes("/opt/axon/libaxon_pjrt.so")
    except Exception as e:
        print(f"ntff hook unavailable: {e}", file=sys.stderr)
        return
    if hook is None:
        return
    import antenv
    mod = types.ModuleType("antenv.axon_hooks")
    mod.get_axon_ntff_profile_hook = lambda: hook
    mod.set_axon_ntff_profile_hook = lambda h: None
    sys.modules["antenv.axon_hooks"] = mod
    antenv.axon_hooks = mod


def kernel(x, conv1_kernel, conv2_kernel, fc1_w, fc1_b, fc2_w, fc2_b):
    from concourse.bass_utils import run_bass_kernel_spmd

    hw = build_host_weights(conv1_kernel, conv2_kernel, fc1_w, fc1_b,
                            fc2_w, fc2_b)

    key = "nc"
    if key not in _CACHE:
        _CACHE[key] = build_bass_kernel()
    nc = _CACHE[key]

    xbf = np.asarray(x, np.float32).reshape(B_TOTAL, 784).astype(BF16)
    shared = {k: hw[k] for k in ("wt1", "wt2", "fc1t", "fc1b", "fc2t", "fc2b", "ident")}
    in_maps = []
    for c in range(N_CORES):
        m = dict(shared)
        m["xt7"] = build_xt7(xbf[c * B_CORE:(c + 1) * B_CORE])
        in_maps.append(m)

    trace = os.environ.get("KERNEL_TRACE", "0") == "1"
    if trace:
        _install_ntff_hook()
    res = run_bass_kernel_spmd(nc, in_maps, core_ids=list(range(N_CORES)),
                               trace=trace)
    if trace and res.exec_time_ns is not None:
        print(f"HW exec time: {res.exec_time_ns} ns", file=sys.stderr)
        _CACHE["exec_time_ns"] = res.exec_time_ns

    outs = []
    for r in res.results:
        yc = r["y"]                          # [128, n_groups, NSUB, 10]
        outs.append(np.ascontiguousarray(
            yc.transpose(1, 2, 0, 3).reshape(B_CORE, 10)))
    return np.concatenate(outs, axis=0)
